# revision 1
# baseline (speedup 1.0000x reference)
"""Trainium2 Bass kernel for nn_BirdModel (LSTM over T=1024, B=256, IN=128, H=64, OUT=100).

Data-parallel over batch across 8 NeuronCores (32 rows each).  Single
recurrence chain per core; per timestep the critical loop is 7 instructions:

    4 matmuls (Whh @ h accumulated onto the precomputed x-projection in PSUM,
               one PSUM bank per gate)
 -> 1 sigmoid over all 4 gates (tanh(g) folded in via tanh z = 2*sigmoid(2z)-1;
       output written gate-blocked at stride 2 with interleaved zeros)
 -> 1 DVE scalar_tensor_tensor   p = (sg - 0.5) * si          [= i*tanh(g)/2]
 -> 1 DVE tensor_tensor_scan     c2[b] = f[b]*c2[b] + p[b]    (c2 == c/2; the
       scan recurrence (d0*state)+d1 is reseeded per batch element by a
       zero multiplier column, so one instruction does reload+update for
       the whole batch)
 -> 1 ACT tanh with scale=2      th = tanh(2*c2) = tanh(c)
 -> 1 DVE tensor_tensor          h = o * th
 -> next step's matmuls.

The x-projection (specs @ W_ih.T) is precomputed chunk-by-chunk (16 steps)
directly into the same PSUM banks the recurrence accumulates into: bf16 specs
staged via GPSIMD cast-DMA, transposed on-device with the DMA xbar.  Biases
ride an augmented ones-row of h (K=65).  Gate order (i, f, o, g), g-gate
weights pre-scaled by 2.
"""

import time
import numpy as np
import ml_dtypes

import concourse.bass as bass
import concourse.mybir as mybir
from concourse.tile import TileContext
from concourse.vector_clock import ScopedClock
from concourse.bass_utils import run_bass_kernel_spmd

B, T, IN, H, OUT = 256, 1024, 128, 64, 100
NCORES = 8
BL = B // NCORES          # 32 batch rows per core
C = 16                    # timesteps per chunk (PSUM bank = 512 fp32 per gate)
NCH = T // C
G4 = 4 * H                # 256

f32 = mybir.dt.float32
bf16 = mybir.dt.bfloat16
AF = mybir.ActivationFunctionType
ALU = mybir.AluOpType

_patched = [False]


def _patch_tile_drain():
    """The walrus build in this environment rejects instructions carrying more
    than one semaphore wait.  Patch the TileContext tail drain to spread its
    waits over single-wait NOPs."""
    if _patched[0]:
        return
    _patched[0] = True

    def _drain_and_barrier(self, tick_clock, wait_clock):
        nc = self.nc
        probe = nc.sync.nop(nofuse=True)
        wait_clock.add_sem_waits(probe.ins, ScopedClock({None: tick_clock.global_clock}))
        si = probe.ins.sync_info
        waits = list(si.on_wait) if si is not None else []
        if waits:
            probe.ins.sync_info = mybir.SyncInfo(on_wait=[waits[0]], on_update=[])
            for w in waits[1:]:
                n = nc.sync.nop(nofuse=True)
                n.ins.sync_info = mybir.SyncInfo(on_wait=[w], on_update=[])
        nc.sync.drain()
        nc.all_engine_barrier()
        assert self.sems is not None
        popped = nc._tile_sem_poison_stack.pop()
        assert popped is self._sem_poison
        nc.clear_and_free_semaphores(list(self.sems.allocated().values()))
        nc.all_engine_barrier()

    TileContext._drain_and_barrier = _drain_and_barrier


def _split_multi_waits(nc):
    """Hoist all-but-one semaphore wait of every instruction onto preceding
    single-wait NOPs (same walrus limitation as above, for the whole program)."""
    ctr = 0
    for f in nc.m.functions:
        for bb in f.blocks:
            out = []
            changed = False
            for inst in bb.instructions:
                si = getattr(inst, "sync_info", None)
                if si is not None and si.on_wait is not None and len(si.on_wait) > 1:
                    waits = list(si.on_wait)
                    for w in waits[:-1]:
                        ctr += 1
                        out.append(mybir.InstNoOp(
                            name=f"I-waitsplit-{ctr}",
                            engine=inst.engine,
                            bass_nofuse=True,
                            sync_info=mybir.SyncInfo(on_wait=[w], on_update=[]),
                        ))
                    inst.sync_info = mybir.SyncInfo(
                        on_wait=[waits[-1]], on_update=list(si.on_update or []))
                    changed = True
                out.append(inst)
            if changed:
                bb.instructions = out
    return ctr


def _fap(t, dims, offset=0):
    """AP over tile `t` keeping its partition dim, with custom free dims
    [[stride, count], ...] (innermost last; negative strides allowed) at an
    element offset."""
    c = t.copy()
    v = c.ap
    pdim = tuple(v[0])
    v.clear()
    v.append(pdim)
    for d in dims:
        v.append(tuple(d))
    c.offset = c.offset + offset
    return c



# Engines whose instruction N+1 cannot start before instruction N completed
# (single pipeline + drain).  PE overlaps fill/drain, so only start-order
# knowledge is inherited there.
_FIFO_ENGINES = {
    mybir.EngineType.Activation,
    mybir.EngineType.DVE,
    mybir.EngineType.Pool,
    mybir.EngineType.SP,
}


def elide_redundant_waits(nc, verbose=False):
    insts = []
    for f in nc.m.functions:
        for bb in f.blocks:
            insts.extend(bb.instructions)

    # 1. identify engine semaphores and their updater streams
    #    sem name -> list of inst indices in update order (None if poisoned)
    upd_stream: dict[str, list[int]] = {}
    poisoned: set[str] = set()
    eng_sems: set[str] = set()
    for idx, inst in enumerate(insts):
        si = getattr(inst, "sync_info", None)
        if si is None or not si.on_update:
            continue
        for u in si.on_update:
            name = u.ant_name
            if name is None:
                continue
            is_eng = any(name.startswith(p) for p in
                         ("Activation", "DVE", "PE", "Pool", "SP"))
            if not is_eng:
                continue
            eng_sems.add(name)
            tname = type(inst).__name__
            if (u.update_mode != "sem-inc" or u.update_value != 1
                    or "Dma" in tname or "DMA" in tname
                    or "EventSemaphore" in tname):
                # non-unit updates, async DMA completions, and barrier
                # set/clears break program-order value attribution
                poisoned.add(name)
                continue
            upd_stream.setdefault(name, []).append(idx)

    # value v of sem S is produced by completion of insts[upd_stream[S][v-1]]
    # (sems start at 0 after the Tile preamble clear).

    # 2. forward pass
    n_drop = 0
    n_kept = 0
    startK: list[dict] = [None] * len(insts)   # knowledge at instruction start
    compK: list[dict] = [None] * len(insts)    # knowledge at completion
    last_on_engine: dict = {}

    def join(a, b):
        if not b:
            return a
        for k, v in b.items():
            if a.get(k, 0) < v:
                a[k] = v
        return a

    for idx, inst in enumerate(insts):
        eng = getattr(inst, "engine", None)
        K: dict = {}
        prev = last_on_engine.get(eng)
        if prev is not None:
            if eng in _FIFO_ENGINES:
                K = dict(compK[prev])
            else:
                K = dict(startK[prev])
        si = getattr(inst, "sync_info", None)
        if si is not None and si.on_wait:
            # split analyzable vs opaque waits
            opaque, cand = [], []
            for w in si.on_wait:
                name = w.ant_name
                if (name is None or name not in eng_sems
                        or name in poisoned
                        or w.wait_mode != "sem-ge-imm"):
                    opaque.append(w)
                else:
                    cand.append(w)
            # fixpoint: a wait is droppable if inherited knowledge plus the
            # closure of the OTHER kept waits implies it (order-independent)
            kept = list(cand)
            changed = True
            while changed:
                changed = False
                for i in range(len(kept)):
                    Ko = dict(K)
                    for j, w2 in enumerate(kept):
                        if j == i:
                            continue
                        Ko[w2.ant_name] = max(Ko.get(w2.ant_name, 0),
                                              w2.wait_value)
                        stream = upd_stream.get(w2.ant_name)
                        if stream is not None and 0 < w2.wait_value <= len(stream):
                            p = stream[w2.wait_value - 1]
                            if compK[p] is not None:
                                join(Ko, compK[p])
                    w = kept[i]
                    if Ko.get(w.ant_name, 0) >= w.wait_value:
                        kept.pop(i)
                        n_drop += 1
                        changed = True
                        break
            n_kept += len(kept)
            for w in kept:
                K[w.ant_name] = max(K.get(w.ant_name, 0), w.wait_value)
                stream = upd_stream.get(w.ant_name)
                if stream is not None and 0 < w.wait_value <= len(stream):
                    p = stream[w.wait_value - 1]
                    if compK[p] is not None:
                        join(K, compK[p])
            new_waits = opaque + kept
            if len(new_waits) != len(si.on_wait):
                inst.sync_info = mybir.SyncInfo(
                    on_wait=new_waits, on_update=list(si.on_update or []))
        startK[idx] = K
        ck = K
        if si is not None and si.on_update:
            ck = dict(K)
            for u in si.on_update:
                name = u.ant_name
                if (name in eng_sems and name not in poisoned
                        and u.update_mode == "sem-inc" and u.update_value == 1):
                    # this completion produces value = position in stream
                    stream = upd_stream[name]
                    # find v: count of updates up to and including idx.
                    # positions are appended in order; use bisect
                    import bisect
                    v = bisect.bisect_right(stream, idx)
                    if ck.get(name, 0) < v:
                        ck[name] = v
        compK[idx] = ck
        if eng is not None:
            last_on_engine[eng] = idx

    if verbose:
        print(f"elide_redundant_waits: dropped {n_drop}, kept {n_kept}, "
              f"poisoned sems: {sorted(poisoned)}")
    return n_drop


def _build_program(split_waits=True):
    _patch_tile_drain()
    nc = bass.Bass("TRN2", target_bir_lowering=False, debug=False)

    specsT_d = nc.dram_tensor("specsT", [IN, T, BL], f32, kind="ExternalInput")
    whhT_d = nc.dram_tensor("whhT", [H + 1, G4], f32, kind="ExternalInput")
    wihT_d = nc.dram_tensor("wihT", [IN, G4], f32, kind="ExternalInput")
    woutb_d = nc.dram_tensor("woutb", [H + 1, OUT], f32, kind="ExternalInput")
    y_d = nc.dram_tensor("y", [BL, OUT], f32, kind="ExternalOutput")

    specsT_ap = specsT_d.ap()

    with TileContext(nc) as tc:
        with tc.tile_pool(name="const", bufs=1) as constp, \
             tc.tile_pool(name="state", bufs=1) as statep, \
             tc.tile_pool(name="sT", bufs=3) as sTp, \
             tc.tile_pool(name="gates", bufs=2, space="PSUM") as gatesp:

            whhT = constp.tile([H + 1, G4], f32)
            wihT = constp.tile([IN, G4], f32)
            woutb = constp.tile([H + 1, OUT], f32)
            nc.sync.dma_start(whhT[:], whhT_d.ap())
            nc.sync.dma_start(wihT[:], wihT_d.ap())
            nc.sync.dma_start(woutb[:], woutb_d.ap())

            h_aug = statep.tile([H + 1, BL], f32)   # rows 0:64 = h, row 64 = 1.0
            nc.vector.memset(h_aug[0:H, :], 0.0)
            nc.vector.memset(h_aug[H:H + 1, :], 1.0)

            # sigmoid output: 4 gate blocks of 2*BL columns; gate k values at
            # columns 2*BL*k + 2*b, odd columns stay zero forever (they feed
            # the scan's reseed multiplier).
            s_t = statep.tile([H, 4 * 2 * BL], f32)
            nc.vector.memset(s_t[:], 0.0)

            # c/2-state ping-pong buffers; invariant in both: c2_b at column
            # 2b+1, p_b staged at column 2b+2, so the scan reads columns
            # [1..65) as the uniform sequence [c2_0, p_0, c2_1, p_1, ...].
            cb0 = statep.tile([H, 2 * BL + 2], f32)
            cb1 = statep.tile([H, 2 * BL + 2], f32)
            cb = [cb0, cb1]
            nc.vector.memset(cb0[:], 0.0)
            nc.vector.memset(cb1[:], 0.0)

            th_t = statep.tile([H, BL], f32)        # tanh(c)

            # per-chunk tile registries
            sT_tiles = {}
            gates_tiles = {}

            def emit_spec_load(ch):
                """One HWDGE DMA: host-pretransposed specsT chunk, f32,
                columns (b, t) b-major."""
                sT_tiles[ch] = sTp.tile([IN, C * BL], f32, tag="sT",
                                        name=f"sT_{ch}")
                nc.sync.dma_start(sT_tiles[ch][:],
                                  specsT_ap[:, ch * C:(ch + 1) * C, :])

            def emit_xp_mm(ch, k):
                """x-projection matmul for gate k of chunk ch: opens the
                accumulation group of PSUM bank k of that chunk's gates tile."""
                if ch not in gates_tiles:
                    # free layout: (gate k, time t, batch b) -- t-major, so the
                    # per-step matmul writes and sigmoid reads are contiguous
                    gates_tiles[ch] = gatesp.tile([H, 4, C, BL], f32, tag="gates",
                                                  name=f"gates_{ch}")
                g = gates_tiles[ch]
                nc.tensor.matmul(g[:, k], wihT[:, k * H:(k + 1) * H], sT_tiles[ch][:],
                                 start=True, stop=False, skip_group_check=True)

            # prologue: specsT for chunks 0 and 1, x-projection for chunk 0
            emit_spec_load(0)
            emit_spec_load(1)
            for k in range(4):
                emit_xp_mm(0, k)

            def emit_mms(g, tl, last):
                for k in range(4):
                    nc.tensor.matmul(_fap(g, [[1, BL]], k * BL * C + tl * BL),
                                     whhT[:, k * H:(k + 1) * H],
                                     h_aug[:], start=False,
                                     stop=last and k == 3,
                                     skip_group_check=True)

            for ch in range(NCH):
                g = gates_tiles[ch]
                for tl in range(C):
                    t = ch * C + tl
                    src = cb[t % 2]
                    dst = cb[1 - t % 2]

                    # sigmoid of all 4 gates; out gate-blocked at stride 2
                    nc.scalar.activation(
                        _fap(s_t, [[2 * BL, 4], [2, BL]]),
                        _fap(g, [[BL * C, 4], [1, BL]], tl * BL),
                        AF.Sigmoid)

                    # p = (sg - 0.5) * si -> src columns 2b+2
                    nc.vector.scalar_tensor_tensor(
                        _fap(src, [[2, BL]], 2),
                        _fap(s_t, [[2, BL]], 3 * 2 * BL), 0.5,
                        _fap(s_t, [[2, BL]]),
                        ALU.subtract, ALU.mult)

                    # c2' = f*c2 + p: one scan over [c2_0, p_0, c2_1, p_1, ...]
                    # with multiplier sequence [0, f_0, 0, f_1, ...] (the zero
                    # reseeds the running state with c2_b before each update).
                    nc.vector.tensor_tensor_scan(
                        _fap(dst, [[1, 2 * BL]]),
                        _fap(s_t, [[1, 2 * BL]], 1 * 2 * BL - 1),
                        _fap(src, [[1, 2 * BL]], 1),
                        0.0, ALU.mult, ALU.add)

                    # th = tanh(2*c2) = tanh(c)
                    nc.scalar.activation(
                        th_t[:], _fap(dst, [[2, BL]], 1),
                        AF.Tanh, scale=2.0)

                    # h = o * th
                    nc.vector.tensor_mul(
                        h_aug[0:H, :],
                        _fap(s_t, [[2, BL]], 2 * 2 * BL),
                        th_t[:])

                    # next step's gate matmuls
                    if tl + 1 < C:
                        emit_mms(g, tl + 1, tl + 1 == C - 1)
                    elif ch + 1 < NCH:
                        emit_mms(gates_tiles[ch + 1], 0, False)

                    # interleaved prefetch for future chunks
                    if tl == 0 and ch + 2 < NCH:
                        emit_spec_load(ch + 2)
                    if tl % 4 == 1 and ch + 1 < NCH:
                        emit_xp_mm(ch + 1, tl // 4)
                del gates_tiles[ch]
                if ch in sT_tiles:
                    del sT_tiles[ch]

        # final projection: y = h.T @ W_out.T + b_out (ones-row supplies bias)
        with tc.tile_pool(name="out", bufs=1) as outp, \
             tc.tile_pool(name="ypsum", bufs=1, space="PSUM") as yp:
            y_ps = yp.tile([BL, OUT], f32)
            nc.tensor.matmul(y_ps[:], h_aug[:], woutb[:], start=True, stop=True)
            y_sb = outp.tile([BL, OUT], f32)
            nc.scalar.copy(y_sb[:], y_ps[:])
            nc.sync.dma_start(y_d.ap(), y_sb[:])

    elide_redundant_waits(nc)
    if split_waits:
        _split_multi_waits(nc)
    return nc


def _prep_weights(W_ih, W_hh, b_ih, b_hh, W_out, b_out):
    # torch gate order (i, f, g, o) -> layout order (i, f, o, g); g scaled by 2
    order = [0, 1, 3, 2]
    bias = (b_ih + b_hh).astype(np.float32)
    whhT = np.zeros((H + 1, G4), dtype=np.float32)
    wihT = np.zeros((IN, G4), dtype=np.float32)
    for kk, blk in enumerate(order):
        scale = 2.0 if blk == 2 else 1.0
        whhT[0:H, kk * H:(kk + 1) * H] = scale * W_hh[blk * H:(blk + 1) * H].T
        whhT[H, kk * H:(kk + 1) * H] = scale * bias[blk * H:(blk + 1) * H]
        wihT[:, kk * H:(kk + 1) * H] = scale * W_ih[blk * H:(blk + 1) * H].T
    woutb = np.zeros((H + 1, OUT), dtype=np.float32)
    woutb[0:H] = W_out.T
    woutb[H] = b_out
    return {
        "whhT": whhT,
        "wihT": wihT,
        "woutb": woutb,
    }


_cached_nc = [None]


def _make_sharded_fn(nc, n_cores):
    """Build the bass2jax multi-core PJRT callable once so bench() can time
    repeated executions without re-tracing."""
    import jax
    from jax.sharding import Mesh, PartitionSpec
    from jax.experimental.shard_map import shard_map
    from concourse import bass2jax

    bass2jax.install_neuronx_cc_hook()
    partition_name = nc.partition_id_tensor.name if nc.partition_id_tensor else None
    in_names, out_names, out_avals, zero_outs = [], [], [], []
    for alloc in nc.m.functions[0].allocations:
        if not isinstance(alloc, mybir.MemoryLocationSet):
            continue
        name = alloc.memorylocations[0].name
        if alloc.kind == "ExternalInput":
            if name != partition_name:
                in_names.append(name)
        elif alloc.kind == "ExternalOutput":
            out_names.append(name)
            shape = tuple(alloc.tensor_shape)
            dtype = mybir.dt.np(alloc.dtype)
            out_avals.append(jax.core.ShapedArray(shape, dtype))
            zero_outs.append(np.zeros(shape, dtype))
    n_params = len(in_names)
    all_in = list(in_names) + list(out_names)
    if partition_name:
        all_in.append(partition_name)

    def _body(*args):
        operands = list(args)
        if partition_name:
            operands.append(bass2jax.partition_id_tensor())
        outs = bass2jax._bass_exec_p.bind(
            *operands, out_avals=tuple(out_avals), in_names=tuple(all_in),
            out_names=tuple(out_names), lowering_input_output_aliases=(),
            sim_require_finite=True, sim_require_nnan=True, nc=nc)
        return tuple(outs)

    devices = jax.devices()[:n_cores]
    mesh = Mesh(np.asarray(devices), ("core",))
    in_specs = (PartitionSpec("core"),) * (n_params + len(out_names))
    out_specs = (PartitionSpec("core"),) * len(out_names)
    fn = jax.jit(shard_map(_body, mesh=mesh, in_specs=in_specs,
                           out_specs=out_specs, check_rep=False),
                 keep_unused=True)
    return fn, in_names, out_names, zero_outs, mesh


def bench(specs, W_ih, W_hh, b_ih, b_hh, W_out, b_out, pipeline=2048, trials=3):
    """Amortized per-execution wall time of the sharded NEFF: issue `pipeline`
    executions back-to-back (device-staged inputs), block once, divide.  This
    measures device execution plus the marginal dispatch cost, excluding the
    fixed per-round-trip tunnel latency."""
    import jax
    from jax.sharding import NamedSharding, PartitionSpec

    specs = np.asarray(specs, dtype=np.float32)
    specsT = np.ascontiguousarray(
        specs.reshape(NCORES, BL, T, IN).transpose(0, 3, 2, 1)
    ).reshape(NCORES * IN, T, BL)
    w = _prep_weights(np.asarray(W_ih, np.float32), np.asarray(W_hh, np.float32),
                      np.asarray(b_ih, np.float32), np.asarray(b_hh, np.float32),
                      np.asarray(W_out, np.float32), np.asarray(b_out, np.float32))
    if _cached_nc[0] is None:
        _cached_nc[0] = _build_program()
    nc = _cached_nc[0]
    fn, in_names, out_names, zero_outs, mesh = _make_sharded_fn(nc, NCORES)
    concat = []
    for name in in_names:
        if name == "specsT":
            concat.append(specsT)
        else:
            concat.append(np.concatenate([w[name]] * NCORES, axis=0))
    concat += [np.zeros((NCORES * z.shape[0], *z.shape[1:]), z.dtype)
               for z in zero_outs]
    sh = NamedSharding(mesh, PartitionSpec("core"))
    staged = [jax.device_put(a, sh) for a in concat]
    out = fn(*staged)
    jax.block_until_ready(out)
    times = []
    for _ in range(trials):
        t0 = time.perf_counter()
        outs = [fn(*staged) for _ in range(pipeline)]
        jax.block_until_ready(outs)
        times.append((time.perf_counter() - t0) / pipeline)
    return min(times) * 1e9


def kernel(specs, W_ih, W_hh, b_ih, b_hh, W_out, b_out, _trace=False):
    specs = np.asarray(specs, dtype=np.float32)
    w = _prep_weights(np.asarray(W_ih, np.float32), np.asarray(W_hh, np.float32),
                      np.asarray(b_ih, np.float32), np.asarray(b_hh, np.float32),
                      np.asarray(W_out, np.float32), np.asarray(b_out, np.float32))
    if _cached_nc[0] is None:
        _cached_nc[0] = _build_program()
    nc = _cached_nc[0]
    in_maps = []
    for core in range(NCORES):
        m = dict(w)
        m["specsT"] = np.ascontiguousarray(
            specs[core * BL:(core + 1) * BL].transpose(2, 1, 0))
        in_maps.append(m)
    res = run_bass_kernel_spmd(nc, in_maps, core_ids=list(range(NCORES)),
                               trace=_trace)
    y = np.concatenate([r["y"] for r in res.results], axis=0)
    if _trace:
        return y, res
    return y



# revision 2
# speedup vs baseline: 1.4101x; 1.4101x over previous
"""Trainium2 Bass kernel for nn_BirdModel (LSTM over T=1024, B=256, IN=128, H=64, OUT=100).

Data-parallel over batch across 8 NeuronCores (32 rows each).  Single
recurrence chain per core; per timestep the critical loop is:

    4 bf16 matmuls (Whh @ h accumulated onto the precomputed x-projection in
               PSUM; gate order i,f,g,o; g-gate weights pre-scaled by 2)
 -> 1 sigmoid over gates i,f,g (tanh(g) folded in via tanh z = 2*sigmoid(2z)-1;
       output written gate-blocked at stride 2 with interleaved zeros); the
       o-gate sigmoid is a second ACT instruction off the critical path,
       written densely in bf16
 -> 1 DVE scalar_tensor_tensor   p = (sg - 0.5) * si          [= i*tanh(g)/2]
 -> 1 DVE tensor_tensor_scan     c2[b] = f[b]*c2[b] + p[b]    (c2 == c/2; the
       scan recurrence (d0*state)+d1 is reseeded per batch element by a
       zero multiplier column, so one instruction does reload+update for
       the whole batch)
 -> 1 ACT tanh with scale=2      th = tanh(2*c2) = tanh(c)    [bf16 out]
 -> 1 DVE tensor_tensor          h = o * th                   [all-bf16, 2x mode]
 -> next step's matmuls.

Everything the PE touches is bf16 (weights, h, specs); PSUM accumulation and
the c-state stay fp32.  The x-projection (specs @ W_ih.T) is precomputed
chunk-by-chunk (16 steps) directly into the same PSUM banks the recurrence
accumulates into.  Biases ride an augmented ones-row of h (K=65).
"""

import time
import numpy as np
import ml_dtypes

import concourse.bass as bass
import concourse.mybir as mybir
from concourse.tile import TileContext
from concourse.vector_clock import ScopedClock
from concourse.bass_utils import run_bass_kernel_spmd

B, T, IN, H, OUT = 256, 1024, 128, 64, 100
NCORES = 8
BL = B // NCORES          # 32 batch rows per core
C = 16                    # timesteps per chunk (PSUM bank = 512 fp32 per gate)
NCH = T // C
G4 = 4 * H                # 256

f32 = mybir.dt.float32
bf16 = mybir.dt.bfloat16
AF = mybir.ActivationFunctionType
ALU = mybir.AluOpType

_patched = [False]


def _patch_tile_drain():
    """The walrus build in this environment rejects instructions carrying more
    than one semaphore wait.  Patch the TileContext tail drain to spread its
    waits over single-wait NOPs."""
    if _patched[0]:
        return
    _patched[0] = True

    def _drain_and_barrier(self, tick_clock, wait_clock):
        nc = self.nc
        probe = nc.sync.nop(nofuse=True)
        wait_clock.add_sem_waits(probe.ins, ScopedClock({None: tick_clock.global_clock}))
        si = probe.ins.sync_info
        waits = list(si.on_wait) if si is not None else []
        if waits:
            probe.ins.sync_info = mybir.SyncInfo(on_wait=[waits[0]], on_update=[])
            for w in waits[1:]:
                n = nc.sync.nop(nofuse=True)
                n.ins.sync_info = mybir.SyncInfo(on_wait=[w], on_update=[])
        nc.sync.drain()
        nc.all_engine_barrier()
        assert self.sems is not None
        popped = nc._tile_sem_poison_stack.pop()
        assert popped is self._sem_poison
        nc.clear_and_free_semaphores(list(self.sems.allocated().values()))
        nc.all_engine_barrier()

    TileContext._drain_and_barrier = _drain_and_barrier


def _split_multi_waits(nc):
    """Hoist all-but-one semaphore wait of every instruction onto preceding
    single-wait NOPs (same walrus limitation as above, for the whole program)."""
    ctr = 0
    for f in nc.m.functions:
        for bb in f.blocks:
            out = []
            changed = False
            for inst in bb.instructions:
                si = getattr(inst, "sync_info", None)
                if si is not None and si.on_wait is not None and len(si.on_wait) > 1:
                    waits = list(si.on_wait)
                    for w in waits[:-1]:
                        ctr += 1
                        out.append(mybir.InstNoOp(
                            name=f"I-waitsplit-{ctr}",
                            engine=inst.engine,
                            bass_nofuse=True,
                            sync_info=mybir.SyncInfo(on_wait=[w], on_update=[]),
                        ))
                    inst.sync_info = mybir.SyncInfo(
                        on_wait=[waits[-1]], on_update=list(si.on_update or []))
                    changed = True
                out.append(inst)
            if changed:
                bb.instructions = out
    return ctr


def _fap(t, dims, offset=0):
    """AP over tile `t` keeping its partition dim, with custom free dims
    [[stride, count], ...] (innermost last; negative strides allowed) at an
    element offset."""
    c = t.copy()
    v = c.ap
    pdim = tuple(v[0])
    v.clear()
    v.append(pdim)
    for d in dims:
        v.append(tuple(d))
    c.offset = c.offset + offset
    return c



# Engines whose instruction N+1 cannot start before instruction N completed
# (single pipeline + drain).  PE overlaps fill/drain, so only start-order
# knowledge is inherited there.
_FIFO_ENGINES = {
    mybir.EngineType.Activation,
    mybir.EngineType.DVE,
    mybir.EngineType.Pool,
    mybir.EngineType.SP,
}


def elide_redundant_waits(nc, verbose=False):
    insts = []
    for f in nc.m.functions:
        for bb in f.blocks:
            insts.extend(bb.instructions)

    # 1. identify engine semaphores and their updater streams
    #    sem name -> list of inst indices in update order (None if poisoned)
    upd_stream: dict[str, list[int]] = {}
    poisoned: set[str] = set()
    eng_sems: set[str] = set()
    for idx, inst in enumerate(insts):
        si = getattr(inst, "sync_info", None)
        if si is None or not si.on_update:
            continue
        for u in si.on_update:
            name = u.ant_name
            if name is None:
                continue
            is_eng = any(name.startswith(p) for p in
                         ("Activation", "DVE", "PE", "Pool", "SP"))
            if not is_eng:
                continue
            eng_sems.add(name)
            tname = type(inst).__name__
            if (u.update_mode != "sem-inc" or u.update_value != 1
                    or "Dma" in tname or "DMA" in tname
                    or "EventSemaphore" in tname):
                # non-unit updates, async DMA completions, and barrier
                # set/clears break program-order value attribution
                poisoned.add(name)
                continue
            upd_stream.setdefault(name, []).append(idx)

    # value v of sem S is produced by completion of insts[upd_stream[S][v-1]]
    # (sems start at 0 after the Tile preamble clear).

    # 2. forward pass
    n_drop = 0
    n_kept = 0
    startK: list[dict] = [None] * len(insts)   # knowledge at instruction start
    compK: list[dict] = [None] * len(insts)    # knowledge at completion
    last_on_engine: dict = {}

    def join(a, b):
        if not b:
            return a
        for k, v in b.items():
            if a.get(k, 0) < v:
                a[k] = v
        return a

    for idx, inst in enumerate(insts):
        eng = getattr(inst, "engine", None)
        K: dict = {}
        prev = last_on_engine.get(eng)
        if prev is not None:
            if eng in _FIFO_ENGINES:
                K = dict(compK[prev])
            else:
                K = dict(startK[prev])
        si = getattr(inst, "sync_info", None)
        if si is not None and si.on_wait:
            # split analyzable vs opaque waits
            opaque, cand = [], []
            for w in si.on_wait:
                name = w.ant_name
                if (name is None or name not in eng_sems
                        or name in poisoned
                        or w.wait_mode != "sem-ge-imm"):
                    opaque.append(w)
                else:
                    cand.append(w)
            # fixpoint: a wait is droppable if inherited knowledge plus the
            # closure of the OTHER kept waits implies it (order-independent)
            kept = list(cand)
            changed = True
            while changed:
                changed = False
                for i in range(len(kept)):
                    Ko = dict(K)
                    for j, w2 in enumerate(kept):
                        if j == i:
                            continue
                        Ko[w2.ant_name] = max(Ko.get(w2.ant_name, 0),
                                              w2.wait_value)
                        stream = upd_stream.get(w2.ant_name)
                        if stream is not None and 0 < w2.wait_value <= len(stream):
                            p = stream[w2.wait_value - 1]
                            if compK[p] is not None:
                                join(Ko, compK[p])
                    w = kept[i]
                    if Ko.get(w.ant_name, 0) >= w.wait_value:
                        kept.pop(i)
                        n_drop += 1
                        changed = True
                        break
            n_kept += len(kept)
            for w in kept:
                K[w.ant_name] = max(K.get(w.ant_name, 0), w.wait_value)
                stream = upd_stream.get(w.ant_name)
                if stream is not None and 0 < w.wait_value <= len(stream):
                    p = stream[w.wait_value - 1]
                    if compK[p] is not None:
                        join(K, compK[p])
            new_waits = opaque + kept
            if len(new_waits) != len(si.on_wait):
                inst.sync_info = mybir.SyncInfo(
                    on_wait=new_waits, on_update=list(si.on_update or []))
        startK[idx] = K
        ck = K
        if si is not None and si.on_update:
            ck = dict(K)
            for u in si.on_update:
                name = u.ant_name
                if (name in eng_sems and name not in poisoned
                        and u.update_mode == "sem-inc" and u.update_value == 1):
                    # this completion produces value = position in stream
                    stream = upd_stream[name]
                    # find v: count of updates up to and including idx.
                    # positions are appended in order; use bisect
                    import bisect
                    v = bisect.bisect_right(stream, idx)
                    if ck.get(name, 0) < v:
                        ck[name] = v
        compK[idx] = ck
        if eng is not None:
            last_on_engine[eng] = idx

    if verbose:
        print(f"elide_redundant_waits: dropped {n_drop}, kept {n_kept}, "
              f"poisoned sems: {sorted(poisoned)}")
    return n_drop


def _build_program(split_waits=True):
    _patch_tile_drain()
    nc = bass.Bass("TRN2", target_bir_lowering=False, debug=False)

    specsT_d = nc.dram_tensor("specsT", [IN, T, BL], bf16, kind="ExternalInput")
    whhT_d = nc.dram_tensor("whhT", [H + 1, G4], bf16, kind="ExternalInput")
    wihT_d = nc.dram_tensor("wihT", [IN, G4], bf16, kind="ExternalInput")
    woutb_d = nc.dram_tensor("woutb", [H + 1, OUT], bf16, kind="ExternalInput")
    y_d = nc.dram_tensor("y", [BL, OUT], f32, kind="ExternalOutput")

    specsT_ap = specsT_d.ap()

    with TileContext(nc) as tc:
        with tc.tile_pool(name="const", bufs=1) as constp, \
             tc.tile_pool(name="state", bufs=1) as statep, \
             tc.tile_pool(name="sT", bufs=3) as sTp, \
             tc.tile_pool(name="gates", bufs=2, space="PSUM") as gatesp:

            whhT = constp.tile([H + 1, G4], bf16)
            wihT = constp.tile([IN, G4], bf16)
            woutb = constp.tile([H + 1, OUT], bf16)
            nc.sync.dma_start(whhT[:], whhT_d.ap())
            nc.sync.dma_start(wihT[:], wihT_d.ap())
            nc.sync.dma_start(woutb[:], woutb_d.ap())

        # h (bf16, dense) with an extra all-ones row so K=65 carries biases
            h_aug = statep.tile([H + 1, BL], bf16)   # rows 0:64 = h, row 64 = 1.0
            nc.vector.memset(h_aug[0:H, :], 0.0)
            nc.vector.memset(h_aug[H:H + 1, :], 1.0)

            # sigmoid output for gates i,f,g: 3 gate blocks of 2*BL columns;
            # gate k values at columns 2*BL*k + 2*b, odd columns stay zero
            # forever (they feed the scan's reseed multiplier).
            s_t = statep.tile([H, 3 * 2 * BL], f32)
            nc.vector.memset(s_t[:], 0.0)

            # o-gate sigmoid, dense bf16 (off the critical path)
            o_t = statep.tile([H, BL], bf16)

            # c/2-state ping-pong buffers; invariant in both: c2_b at column
            # 2b+1, p_b staged at column 2b+2, so the scan reads columns
            # [1..65) as the uniform sequence [c2_0, p_0, c2_1, p_1, ...].
            cb0 = statep.tile([H, 2 * BL + 2], f32)
            cb1 = statep.tile([H, 2 * BL + 2], f32)
            cb = [cb0, cb1]
            nc.vector.memset(cb0[:], 0.0)
            nc.vector.memset(cb1[:], 0.0)

            th_t = statep.tile([H, BL], bf16)        # tanh(c), dense bf16

            # per-chunk tile registries
            sT_tiles = {}
            gates_tiles = {}

            def emit_spec_load(ch):
                """One HWDGE DMA: host-pretransposed specsT chunk, bf16,
                columns (b, t) b-major."""
                sT_tiles[ch] = sTp.tile([IN, C * BL], bf16, tag="sT",
                                        name=f"sT_{ch}")
                nc.sync.dma_start(sT_tiles[ch][:],
                                  specsT_ap[:, ch * C:(ch + 1) * C, :])

            def emit_xp_mm(ch, k):
                """x-projection matmul for gate k of chunk ch: opens the
                accumulation group of PSUM bank k of that chunk's gates tile."""
                if ch not in gates_tiles:
                    # free layout: (gate k, time t, batch b) -- t-major, so the
                    # per-step matmul writes and sigmoid reads are contiguous
                    gates_tiles[ch] = gatesp.tile([H, 4, C, BL], f32, tag="gates",
                                                  name=f"gates_{ch}")
                g = gates_tiles[ch]
                nc.tensor.matmul(g[:, k], wihT[:, k * H:(k + 1) * H], sT_tiles[ch][:],
                                 start=True, stop=False, skip_group_check=True)

            # prologue: specsT for chunks 0 and 1, x-projection for chunk 0
            emit_spec_load(0)
            emit_spec_load(1)
            for k in range(4):
                emit_xp_mm(0, k)

            def emit_mms(g, tl, last):
                for k in range(4):
                    nc.tensor.matmul(_fap(g, [[1, BL]], k * BL * C + tl * BL),
                                     whhT[:, k * H:(k + 1) * H],
                                     h_aug[:], start=False,
                                     stop=last and k == 3,
                                     skip_group_check=True)

            for ch in range(NCH):
                g = gates_tiles[ch]
                for tl in range(C):
                    t = ch * C + tl
                    src = cb[t % 2]
                    dst = cb[1 - t % 2]

                    # sigmoid of gates i,f,g; out gate-blocked at stride 2
                    nc.scalar.activation(
                        _fap(s_t, [[2 * BL, 3], [2, BL]]),
                        _fap(g, [[BL * C, 3], [1, BL]], tl * BL),
                        AF.Sigmoid)

                    # o-gate sigmoid, dense bf16 (not on the critical path:
                    # its consumer is the h-mult after the tanh)
                    nc.scalar.activation(
                        o_t[:],
                        _fap(g, [[1, BL]], 3 * BL * C + tl * BL),
                        AF.Sigmoid)

                    # p = (sg - 0.5) * si -> src columns 2b+2
                    nc.vector.scalar_tensor_tensor(
                        _fap(src, [[2, BL]], 2),
                        _fap(s_t, [[2, BL]], 2 * 2 * BL), 0.5,
                        _fap(s_t, [[2, BL]]),
                        ALU.subtract, ALU.mult)

                    # c2' = f*c2 + p: one scan over [c2_0, p_0, c2_1, p_1, ...]
                    # with multiplier sequence [0, f_0, 0, f_1, ...] (the zero
                    # reseeds the running state with c2_b before each update).
                    nc.vector.tensor_tensor_scan(
                        _fap(dst, [[1, 2 * BL]]),
                        _fap(s_t, [[1, 2 * BL]], 1 * 2 * BL - 1),
                        _fap(src, [[1, 2 * BL]], 1),
                        0.0, ALU.mult, ALU.add)

                    # th = tanh(2*c2) = tanh(c), bf16 out
                    nc.scalar.activation(
                        th_t[:], _fap(dst, [[2, BL]], 1),
                        AF.Tanh, scale=2.0)

                    # h = o * th  (all-bf16 dense: DVE 2x mode)
                    nc.vector.tensor_mul(
                        h_aug[0:H, :], o_t[:], th_t[:])

                    # next step's gate matmuls
                    if tl + 1 < C:
                        emit_mms(g, tl + 1, tl + 1 == C - 1)
                    elif ch + 1 < NCH:
                        emit_mms(gates_tiles[ch + 1], 0, False)

                    # interleaved prefetch for future chunks
                    if tl == 0 and ch + 2 < NCH:
                        emit_spec_load(ch + 2)
                    if tl % 4 == 1 and ch + 1 < NCH:
                        emit_xp_mm(ch + 1, tl // 4)
                del gates_tiles[ch]
                if ch in sT_tiles:
                    del sT_tiles[ch]

        # final projection: y = h.T @ W_out.T + b_out (ones-row supplies bias)
        with tc.tile_pool(name="out", bufs=1) as outp, \
             tc.tile_pool(name="ypsum", bufs=1, space="PSUM") as yp:
            y_ps = yp.tile([BL, OUT], f32)
            nc.tensor.matmul(y_ps[:], h_aug[:], woutb[:], start=True, stop=True)
            y_sb = outp.tile([BL, OUT], f32)
            nc.scalar.copy(y_sb[:], y_ps[:])
            nc.sync.dma_start(y_d.ap(), y_sb[:])

    elide_redundant_waits(nc)
    if split_waits:
        _split_multi_waits(nc)
    return nc


def _prep_weights(W_ih, W_hh, b_ih, b_hh, W_out, b_out):
    # gate order (i, f, g, o) == torch LSTMCell order; g-gate scaled by 2 so
    # sigmoid(2x) encodes tanh(x) = 2*sigmoid(2x) - 1
    bias = (b_ih + b_hh).astype(np.float32)
    whhT = np.zeros((H + 1, G4), dtype=np.float32)
    wihT = np.zeros((IN, G4), dtype=np.float32)
    for kk in range(4):
        scale = 2.0 if kk == 2 else 1.0
        whhT[0:H, kk * H:(kk + 1) * H] = scale * W_hh[kk * H:(kk + 1) * H].T
        whhT[H, kk * H:(kk + 1) * H] = scale * bias[kk * H:(kk + 1) * H]
        wihT[:, kk * H:(kk + 1) * H] = scale * W_ih[kk * H:(kk + 1) * H].T
    woutb = np.zeros((H + 1, OUT), dtype=np.float32)
    woutb[0:H] = W_out.T
    woutb[H] = b_out
    return {
        "whhT": whhT.astype(ml_dtypes.bfloat16),
        "wihT": wihT.astype(ml_dtypes.bfloat16),
        "woutb": woutb.astype(ml_dtypes.bfloat16),
    }


_cached_nc = [None]


def _make_sharded_fn(nc, n_cores):
    """Build the bass2jax multi-core PJRT callable once so bench() can time
    repeated executions without re-tracing."""
    import jax
    from jax.sharding import Mesh, PartitionSpec
    from jax.experimental.shard_map import shard_map
    from concourse import bass2jax

    bass2jax.install_neuronx_cc_hook()
    partition_name = nc.partition_id_tensor.name if nc.partition_id_tensor else None
    in_names, out_names, out_avals, zero_outs = [], [], [], []
    for alloc in nc.m.functions[0].allocations:
        if not isinstance(alloc, mybir.MemoryLocationSet):
            continue
        name = alloc.memorylocations[0].name
        if alloc.kind == "ExternalInput":
            if name != partition_name:
                in_names.append(name)
        elif alloc.kind == "ExternalOutput":
            out_names.append(name)
            shape = tuple(alloc.tensor_shape)
            dtype = mybir.dt.np(alloc.dtype)
            out_avals.append(jax.core.ShapedArray(shape, dtype))
            zero_outs.append(np.zeros(shape, dtype))
    n_params = len(in_names)
    all_in = list(in_names) + list(out_names)
    if partition_name:
        all_in.append(partition_name)

    def _body(*args):
        operands = list(args)
        if partition_name:
            operands.append(bass2jax.partition_id_tensor())
        outs = bass2jax._bass_exec_p.bind(
            *operands, out_avals=tuple(out_avals), in_names=tuple(all_in),
            out_names=tuple(out_names), lowering_input_output_aliases=(),
            sim_require_finite=True, sim_require_nnan=True, nc=nc)
        return tuple(outs)

    devices = jax.devices()[:n_cores]
    mesh = Mesh(np.asarray(devices), ("core",))
    in_specs = (PartitionSpec("core"),) * (n_params + len(out_names))
    out_specs = (PartitionSpec("core"),) * len(out_names)
    fn = jax.jit(shard_map(_body, mesh=mesh, in_specs=in_specs,
                           out_specs=out_specs, check_rep=False),
                 keep_unused=True)
    return fn, in_names, out_names, zero_outs, mesh


def bench(specs, W_ih, W_hh, b_ih, b_hh, W_out, b_out, pipeline=2048, trials=3):
    """Amortized per-execution wall time of the sharded NEFF: issue `pipeline`
    executions back-to-back (device-staged inputs), block once, divide.  This
    measures device execution plus the marginal dispatch cost, excluding the
    fixed per-round-trip tunnel latency."""
    import jax
    from jax.sharding import NamedSharding, PartitionSpec

    specs = np.asarray(specs, dtype=np.float32)
    specsT = np.ascontiguousarray(
        specs.reshape(NCORES, BL, T, IN).transpose(0, 3, 2, 1)
    ).astype(ml_dtypes.bfloat16).reshape(NCORES * IN, T, BL)
    w = _prep_weights(np.asarray(W_ih, np.float32), np.asarray(W_hh, np.float32),
                      np.asarray(b_ih, np.float32), np.asarray(b_hh, np.float32),
                      np.asarray(W_out, np.float32), np.asarray(b_out, np.float32))
    if _cached_nc[0] is None:
        _cached_nc[0] = _build_program()
    nc = _cached_nc[0]
    fn, in_names, out_names, zero_outs, mesh = _make_sharded_fn(nc, NCORES)
    concat = []
    for name in in_names:
        if name == "specsT":
            concat.append(specsT)
        else:
            concat.append(np.concatenate([w[name]] * NCORES, axis=0))
    concat += [np.zeros((NCORES * z.shape[0], *z.shape[1:]), z.dtype)
               for z in zero_outs]
    sh = NamedSharding(mesh, PartitionSpec("core"))
    staged = [jax.device_put(a, sh) for a in concat]
    out = fn(*staged)
    jax.block_until_ready(out)
    times = []
    for _ in range(trials):
        t0 = time.perf_counter()
        outs = [fn(*staged) for _ in range(pipeline)]
        jax.block_until_ready(outs)
        times.append((time.perf_counter() - t0) / pipeline)
    return min(times) * 1e9


def kernel(specs, W_ih, W_hh, b_ih, b_hh, W_out, b_out, _trace=False):
    specs = np.asarray(specs, dtype=np.float32)
    w = _prep_weights(np.asarray(W_ih, np.float32), np.asarray(W_hh, np.float32),
                      np.asarray(b_ih, np.float32), np.asarray(b_hh, np.float32),
                      np.asarray(W_out, np.float32), np.asarray(b_out, np.float32))
    if _cached_nc[0] is None:
        _cached_nc[0] = _build_program()
    nc = _cached_nc[0]
    in_maps = []
    for core in range(NCORES):
        m = dict(w)
        m["specsT"] = np.ascontiguousarray(
            specs[core * BL:(core + 1) * BL].transpose(2, 1, 0)
        ).astype(ml_dtypes.bfloat16)
        in_maps.append(m)
    res = run_bass_kernel_spmd(nc, in_maps, core_ids=list(range(NCORES)),
                               trace=_trace)
    y = np.concatenate([r["y"] for r in res.results], axis=0)
    if _trace:
        return y, res
    return y
